# revision 11
# baseline (speedup 1.0000x reference)
"""Trainium2 Bass kernel for nn_Beta_LR_41308995453190.

Network (per (b, o) pair):
  - 13 segment means over the L axis of hidden[b, o] (ragged boundaries
    from idx[b]): 10 context segments, question, option, whole-context.
  - beta-param projection e = 1 + relu(x @ Wp + bp), split a/b.
  - three attention pools (intersection over segments, renew over
    (segment, intersection) pairs, union over inverted renewed params).
  - classify head: concat 8 beta embeddings -> relu(@Wl0 + bl0) -> @Wl + bl.

Sharding: data-parallel over the batch dim B=8 (one batch per NeuronCore),
weights replicated. Segment means are computed as mask matmuls: the host
bakes the ragged boundaries + 1/count into a [L, 13] mask per batch, and
the device contracts hidden tiles against the mask on the tensor engine,
producing the segment means directly in transposed (feature-major) layout.
The whole head then runs in that transposed layout so the softmaxes over
segments are free-axis reductions.
"""

import numpy as np

try:
    import concourse.bass as bass
except ImportError:
    import sys

    sys.path.insert(0, "/opt/trn_rl_repo")
    import concourse.bass as bass

import concourse.tile as tile
from concourse import mybir
from concourse.bass_utils import run_bass_kernel_spmd

F32 = mybir.dt.float32
AX = mybir.AxisListType.X
OP = mybir.AluOpType
AF = mybir.ActivationFunctionType

B, O, L, E = 8, 4, 1024, 1024
BETA = 512
NSEG = 12
NK = 13  # 10 ctx + q + o + allc
P = 128
T = L // P  # 8 L-tiles per option
NCOL = O * NK  # 52


# ---------------------------------------------------------------------------
# Workaround: this neuronxcc walrus build rejects more than one sem wait per
# TPB instruction ("Too many sync wait commands"). Hoist excess waits onto
# no-op instructions inserted immediately before the offending instruction on
# the same engine — the engine blocks at each nop until its condition holds,
# which is semantically identical to multiple waits on one instruction.
# ---------------------------------------------------------------------------
def _split_excess_waits(nc, max_waits=1):
    scratch_bb = nc.cur_bb.bb
    n_split = 0
    for f in nc.m.functions:
        for bb in f.blocks:
            new_list = []
            for ins in bb.instructions:
                si = ins.sync_info
                waits = list(si.on_wait) if si and si.on_wait else []
                if len(waits) > max_waits:
                    n_split += 1
                    for w in waits[: len(waits) - max_waits]:
                        carrier = nc.engines[ins.engine].drain(fusable=False).ins
                        scratch_bb.instructions.remove(carrier)
                        carrier.sync_info = mybir.SyncInfo(
                            on_wait=[w], on_update=[]
                        )
                        new_list.append(carrier)
                    si.on_wait = waits[len(waits) - max_waits :]
                new_list.append(ins)
            bb.instructions[:] = new_list
    if n_split:
        print(f"[kernel] split waits on {n_split} instructions")


def _build_nc(debug=False):
    nc = bass.Bass("TRN2", target_bir_lowering=False)

    hid_d = nc.dram_tensor("hidden", [O, L, E], F32, kind="ExternalInput")
    mask_d = nc.dram_tensor("maskt", [P, T, NK], F32, kind="ExternalInput")
    wp_d = nc.dram_tensor("wp", [P, 8, 1024], F32, kind="ExternalInput")
    wa0_d = nc.dram_tensor("wa0", [P, 8, 512], F32, kind="ExternalInput")
    wa_d = nc.dram_tensor("wa", [P, 4, 512], F32, kind="ExternalInput")
    wl0_d = nc.dram_tensor("wl0", [P, 32, 512], F32, kind="ExternalInput")
    wl_d = nc.dram_tensor("wl", [P, 4], F32, kind="ExternalInput")
    bias_d = nc.dram_tensor("biases", [P, 21], F32, kind="ExternalInput")
    out_d = nc.dram_tensor("out", [1, O], F32, kind="ExternalOutput")

    with tile.TileContext(nc) as tc:
        with (
            tc.tile_pool(name="const", bufs=1) as const,
            tc.tile_pool(name="hidp2", bufs=2) as hidp2,
            tc.tile_pool(name="hidp1", bufs=1) as hidp1,
            tc.tile_pool(name="act", bufs=1) as act,
            tc.tile_pool(name="tmp", bufs=3) as tmp,
            tc.tile_pool(name="pseg", bufs=4, space="PSUM") as pseg,
            tc.tile_pool(name="pmm", bufs=3, space="PSUM") as pmm,
        ):
            # ---- constants (mask/bias first: needed by the seg matmuls)
            mask_sb = const.tile([P, T, NK], F32)
            nc.sync.dma_start(out=mask_sb, in_=mask_d[:])
            bias_sb = const.tile([P, 21], F32)
            nc.sync.dma_start(out=bias_sb, in_=bias_d[:])
            wp_sb = const.tile([P, 8, 1024], F32)
            nc.sync.dma_start(out=wp_sb, in_=wp_d[:])
            wa0_sb = const.tile([P, 8, 512], F32)
            nc.sync.dma_start(out=wa0_sb, in_=wa0_d[:])
            wa_sb = const.tile([P, 4, 512], F32)
            nc.sync.dma_start(out=wa_sb, in_=wa_d[:])
            wl_sb = const.tile([P, 4], F32)
            nc.sync.dma_start(out=wl_sb, in_=wl_d[:])

            def bcol(i):
                return bias_sb[:, i : i + 1]

            # ---- segment means, transposed: xT[c, o, k] = mean over seg k
            # of hidden[o, :, c*128:(c+1)*128]
            # PSUM accumulation-group constraint: a matmul with start=True
            # clears the has-written bits of its whole PSUM bank, so groups
            # sharing a bank must not interleave. Keep each (o, c) group's
            # matmuls consecutive: load all 8 L-tiles of an option, then
            # accumulate chunk by chunk.
            xT = act.tile([P, 8, O, NK], F32)
            for o in range(O):
                htiles = []
                for t in range(T):
                    pool = hidp2 if t < 6 else hidp1
                    htile = pool.tile([P, E], F32, tag=f"htile{t}")
                    nc.sync.dma_start(
                        out=htile, in_=hid_d[o, t * P : (t + 1) * P, :]
                    )
                    htiles.append(htile)
                ps = pseg.tile([P, 8, NK], F32, tag="ps_seg")
                for c in range(8):
                    for t in range(T):
                        nc.tensor.matmul(
                            out=ps[:, c, :],
                            lhsT=htiles[t][:, c * P : (c + 1) * P],
                            rhs=mask_sb[:, t, :],
                            start=(t == 0),
                            stop=(t == T - 1),
                        )
                nc.scalar.copy(out=xT[:, :, o, :], in_=ps[:, :, :])

            # ---- wl0 DMA last: only needed by the final head
            wl0_sb = const.tile([P, 32, 512], F32)
            nc.sync.dma_start(out=wl0_sb, in_=wl0_d[:])

            # ---- projection: eT = max(Wp^T @ x + (bp + 1), 1)
            # eT[m, o, k]: feature chunk m (0..3 -> a, 4..7 -> b)
            eT = act.tile([P, 8, O, NK], F32)
            for m in range(8):
                pe = pmm.tile([P, NCOL], F32, tag="pmm")
                for c in range(8):
                    nc.tensor.matmul(
                        out=pe,
                        lhsT=wp_sb[:, c, m * P : (m + 1) * P],
                        rhs=xT[:, c, :, :],
                        start=(c == 0),
                        stop=(c == 7),
                    )
                nc.vector.tensor_scalar(
                    out=eT[:, m, :, :],
                    in0=pe[:, :],
                    scalar1=bcol(m),
                    scalar2=1.0,
                    op0=OP.add,
                    op1=OP.max,
                )

            # ---- pool 1 (intersection): h1 = relu(Wa0^T @ e + ba0)
            h1T = act.tile([P, 4, NCOL], F32)
            for m in range(4):
                ph = pmm.tile([P, NCOL], F32, tag="pmm")
                for c in range(8):
                    nc.tensor.matmul(
                        out=ph,
                        lhsT=wa0_sb[:, c, m * P : (m + 1) * P],
                        rhs=eT[:, c, :, :],
                        start=(c == 0),
                        stop=(c == 7),
                    )
                nc.vector.tensor_scalar(
                    out=h1T[:, m, :],
                    in0=ph[:, :],
                    scalar1=bcol(8 + m),
                    scalar2=0.0,
                    op0=OP.add,
                    op1=OP.max,
                )

            # l1 = Wa^T @ h1 + ba  (shared by pool 1 softmax and renew pairs)
            l1T = act.tile([P, 4, O, NK], F32)
            for m in range(4):
                pl = pmm.tile([P, NCOL], F32, tag="pmm")
                for c in range(4):
                    nc.tensor.matmul(
                        out=pl,
                        lhsT=wa_sb[:, c, m * P : (m + 1) * P],
                        rhs=h1T[:, c, :],
                        start=(c == 0),
                        stop=(c == 3),
                    )
                nc.vector.tensor_scalar_add(
                    out=l1T[:, m, :, :], in0=pl[:, :], scalar1=bcol(12 + m)
                )

            # pool 1 softmax over the 10 ctx segments + weighted reduce
            # cat2[c, o] = [ia; ib] per option (input of the renew pair pool)
            cat2 = act.tile([P, 8, O], F32)
            for m in range(4):
                lsl = l1T[:, m, :, 0:10]
                mx = tmp.tile([P, O], F32, tag="mx")
                nc.vector.reduce_max(mx, lsl, axis=AX)
                d = tmp.tile([P, O, 10], F32, tag="d")
                nc.vector.tensor_tensor(
                    out=d, in0=lsl, in1=mx.broadcast_to([P, O, 10]), op=OP.subtract
                )
                w = tmp.tile([P, O, 10], F32, tag="w")
                nc.scalar.activation(out=w, in_=d, func=AF.Exp)
                s = tmp.tile([P, O], F32, tag="s")
                nc.vector.reduce_sum(s, w, axis=AX)
                r = tmp.tile([P, O], F32, tag="r")
                nc.vector.reciprocal(out=r, in_=s)
                wn = tmp.tile([P, O, 10], F32, tag="wn")
                nc.vector.tensor_tensor(
                    out=wn, in0=w, in1=r.broadcast_to([P, O, 10]), op=OP.mult
                )
                wa_t = tmp.tile([P, O, 10], F32, tag="wa_t")
                nc.vector.tensor_tensor(
                    out=wa_t, in0=wn, in1=eT[:, m, :, 0:10], op=OP.mult
                )
                nc.vector.reduce_sum(cat2[:, m, :], wa_t, axis=AX)
                wb_t = tmp.tile([P, O, 10], F32, tag="wb_t")
                nc.vector.tensor_tensor(
                    out=wb_t, in0=wn, in1=eT[:, 4 + m, :, 0:10], op=OP.mult
                )
                nc.vector.reduce_sum(cat2[:, 4 + m, :], wb_t, axis=AX)

            # ---- renew: pair each ctx segment with the intersection.
            # h2/l2 for the intersection element (shared across segments)
            h2T = act.tile([P, 4, O], F32)
            for m in range(4):
                p2 = pmm.tile([P, O], F32, tag="pmm")
                for c in range(8):
                    nc.tensor.matmul(
                        out=p2,
                        lhsT=wa0_sb[:, c, m * P : (m + 1) * P],
                        rhs=cat2[:, c, :],
                        start=(c == 0),
                        stop=(c == 7),
                    )
                nc.vector.tensor_scalar(
                    out=h2T[:, m, :],
                    in0=p2[:, :],
                    scalar1=bcol(8 + m),
                    scalar2=0.0,
                    op0=OP.add,
                    op1=OP.max,
                )
            l2T = act.tile([P, 4, O], F32)
            for m in range(4):
                p3 = pmm.tile([P, O], F32, tag="pmm")
                for c in range(4):
                    nc.tensor.matmul(
                        out=p3,
                        lhsT=wa_sb[:, c, m * P : (m + 1) * P],
                        rhs=h2T[:, c, :],
                        start=(c == 0),
                        stop=(c == 3),
                    )
                nc.vector.tensor_scalar_add(
                    out=l2T[:, m, :], in0=p3[:, :], scalar1=bcol(12 + m)
                )

            # pair softmax([l1[k], l2]) -> na/nb; store reciprocals (union input)
            raT = act.tile([P, 4, O, 10], F32)
            rbT = act.tile([P, 4, O, 10], F32)
            for m in range(4):
                l1s = l1T[:, m, :, 0:10]
                l2b = l2T[:, m, :].broadcast_to([P, O, 10])
                mxp = tmp.tile([P, O, 10], F32, tag="mxp")
                nc.vector.tensor_tensor(out=mxp, in0=l1s, in1=l2b, op=OP.max)
                d1 = tmp.tile([P, O, 10], F32, tag="d1")
                nc.vector.tensor_tensor(out=d1, in0=l1s, in1=mxp, op=OP.subtract)
                e1 = tmp.tile([P, O, 10], F32, tag="e1")
                nc.scalar.activation(out=e1, in_=d1, func=AF.Exp)
                d2 = tmp.tile([P, O, 10], F32, tag="d2")
                nc.vector.tensor_tensor(out=d2, in0=l2b, in1=mxp, op=OP.subtract)
                e2 = tmp.tile([P, O, 10], F32, tag="e2")
                nc.scalar.activation(out=e2, in_=d2, func=AF.Exp)
                s12 = tmp.tile([P, O, 10], F32, tag="s12")
                nc.vector.tensor_tensor(out=s12, in0=e1, in1=e2, op=OP.add)
                rs = tmp.tile([P, O, 10], F32, tag="rs")
                nc.vector.reciprocal(out=rs, in_=s12)
                for half, dst in ((0, raT), (1, rbT)):
                    t1 = tmp.tile([P, O, 10], F32, tag="t1")
                    nc.vector.tensor_tensor(
                        out=t1, in0=e1, in1=eT[:, half * 4 + m, :, 0:10], op=OP.mult
                    )
                    t2 = tmp.tile([P, O, 10], F32, tag="t2")
                    nc.vector.tensor_tensor(
                        out=t2,
                        in0=e2,
                        in1=cat2[:, half * 4 + m, :].broadcast_to([P, O, 10]),
                        op=OP.mult,
                    )
                    t3 = tmp.tile([P, O, 10], F32, tag="t3")
                    nc.vector.tensor_tensor(out=t3, in0=t1, in1=t2, op=OP.add)
                    nv = tmp.tile([P, O, 10], F32, tag="nv")
                    nc.vector.tensor_tensor(out=nv, in0=t3, in1=rs, op=OP.mult)
                    nc.vector.reciprocal(out=dst[:, m, :, :], in_=nv)

            # ---- union pool over segments of [1/na; 1/nb]
            h3T = act.tile([P, 4, O, 10], F32)
            for m in range(4):
                p4 = pmm.tile([P, O * 10], F32, tag="pmm")
                for c in range(8):
                    src = raT if c < 4 else rbT
                    nc.tensor.matmul(
                        out=p4,
                        lhsT=wa0_sb[:, c, m * P : (m + 1) * P],
                        rhs=src[:, c % 4, :, :],
                        start=(c == 0),
                        stop=(c == 7),
                    )
                nc.vector.tensor_scalar(
                    out=h3T[:, m, :, :],
                    in0=p4[:, :],
                    scalar1=bcol(8 + m),
                    scalar2=0.0,
                    op0=OP.add,
                    op1=OP.max,
                )
            l3T = act.tile([P, 4, O, 10], F32)
            for m in range(4):
                p5 = pmm.tile([P, O * 10], F32, tag="pmm")
                for c in range(4):
                    nc.tensor.matmul(
                        out=p5,
                        lhsT=wa_sb[:, c, m * P : (m + 1) * P],
                        rhs=h3T[:, c, :, :],
                        start=(c == 0),
                        stop=(c == 3),
                    )
                nc.vector.tensor_scalar_add(
                    out=l3T[:, m, :, :], in0=p5[:, :], scalar1=bcol(12 + m)
                )

            # union softmax + weighted reduce + invert back -> catF chunks 0..7
            # catF[kc, o]: 32 feature chunks of the classify-head input
            catF = act.tile([P, 32, O], F32)
            for m in range(4):
                l3s = l3T[:, m, :, :]
                mx3 = tmp.tile([P, O], F32, tag="mx3")
                nc.vector.reduce_max(mx3, l3s, axis=AX)
                d3 = tmp.tile([P, O, 10], F32, tag="d3")
                nc.vector.tensor_tensor(
                    out=d3, in0=l3s, in1=mx3.broadcast_to([P, O, 10]), op=OP.subtract
                )
                w3 = tmp.tile([P, O, 10], F32, tag="w3")
                nc.scalar.activation(out=w3, in_=d3, func=AF.Exp)
                s3 = tmp.tile([P, O], F32, tag="s3")
                nc.vector.reduce_sum(s3, w3, axis=AX)
                r3 = tmp.tile([P, O], F32, tag="r3")
                nc.vector.reciprocal(out=r3, in_=s3)
                wn3 = tmp.tile([P, O, 10], F32, tag="wn3")
                nc.vector.tensor_tensor(
                    out=wn3, in0=w3, in1=r3.broadcast_to([P, O, 10]), op=OP.mult
                )
                for half, src in ((0, raT), (1, rbT)):
                    tu = tmp.tile([P, O, 10], F32, tag="tu")
                    nc.vector.tensor_tensor(
                        out=tu, in0=wn3, in1=src[:, m, :, :], op=OP.mult
                    )
                    su = tmp.tile([P, O], F32, tag="su")
                    nc.vector.reduce_sum(su, tu, axis=AX)
                    nc.vector.reciprocal(out=catF[:, half * 4 + m, :], in_=su)

            # catF chunks 8..31: a_ac, b_ac, a_o, b_o, a_q, b_q from eT
            for j, (half, k) in enumerate(
                ((0, 12), (1, 12), (0, 11), (1, 11), (0, 10), (1, 10))
            ):
                nc.gpsimd.tensor_copy(
                    out=catF[:, 8 + j * 4 : 12 + j * 4, :],
                    in_=eT[:, half * 4 : half * 4 + 4, :, k],
                )

            # ---- classify head
            hfT = act.tile([P, 4, O], F32)
            for m in range(4):
                pf = pmm.tile([P, O], F32, tag="pmm")
                for kc in range(32):
                    nc.tensor.matmul(
                        out=pf,
                        lhsT=wl0_sb[:, kc, m * P : (m + 1) * P],
                        rhs=catF[:, kc, :],
                        start=(kc == 0),
                        stop=(kc == 31),
                    )
                nc.vector.tensor_scalar(
                    out=hfT[:, m, :],
                    in0=pf[:, :],
                    scalar1=bcol(16 + m),
                    scalar2=0.0,
                    op0=OP.add,
                    op1=OP.max,
                )

            po = pmm.tile([1, O], F32, tag="pmm")
            for c in range(4):
                nc.tensor.matmul(
                    out=po,
                    lhsT=wl_sb[:, c : c + 1],
                    rhs=hfT[:, c, :],
                    start=(c == 0),
                    stop=(c == 3),
                )
            out_sb = act.tile([1, O], F32)
            nc.vector.tensor_scalar_add(
                out=out_sb, in0=po[:, :], scalar1=bias_sb[0:1, 20:21]
            )
            nc.sync.dma_start(out=out_d[:], in_=out_sb)

            if debug:
                for name, t in (
                    ("xT", xT),
                    ("eT", eT),
                    ("h1T", h1T),
                    ("l1T", l1T),
                    ("cat2", cat2),
                    ("raT", raT),
                    ("rbT", rbT),
                    ("catF", catF),
                    ("hfT", hfT),
                ):
                    d = nc.dram_tensor(
                        "dbg_" + name, list(t.shape), F32, kind="ExternalOutput"
                    )
                    nc.sync.dma_start(out=d[:], in_=t)

    _split_excess_waits(nc)
    return nc


_NC = None


def _get_nc():
    global _NC
    if _NC is None:
        _NC = _build_nc()
    return _NC


def _prep_inputs(hidden, idx, Wp, bp, Wa0, ba0, Wa, ba, Wl0, bl0, Wl, bl):
    hidden = np.ascontiguousarray(np.asarray(hidden, dtype=np.float32))
    idx = np.asarray(idx).astype(np.int64)

    f32 = lambda a: np.ascontiguousarray(np.asarray(a, dtype=np.float32))
    Wp, bp = f32(Wp), f32(bp)
    Wa0, ba0 = f32(Wa0), f32(ba0)
    Wa, ba = f32(Wa), f32(ba)
    Wl0, bl0 = f32(Wl0), f32(bl0)
    Wl, bl = f32(Wl), f32(bl)

    wp_t = np.ascontiguousarray(Wp.reshape(8, P, 1024).transpose(1, 0, 2))
    wa0_t = np.ascontiguousarray(Wa0.reshape(8, P, 512).transpose(1, 0, 2))
    wa_t = np.ascontiguousarray(Wa.reshape(4, P, 512).transpose(1, 0, 2))
    wl0_t = np.ascontiguousarray(Wl0.reshape(32, P, 512).transpose(1, 0, 2))
    wl_t = np.ascontiguousarray(Wl.reshape(4, P).transpose(1, 0))

    biases = np.zeros((P, 21), dtype=np.float32)
    biases[:, 0:8] = (bp + 1.0).reshape(8, P).T
    biases[:, 8:12] = ba0.reshape(4, P).T
    biases[:, 12:16] = ba.reshape(4, P).T
    biases[:, 16:20] = bl0.reshape(4, P).T
    biases[:, 20] = bl[0]

    in_maps = []
    for b in range(B):
        m = np.zeros((L, NK), dtype=np.float32)
        ib = idx[b]
        starts = [1] + [int(ib[k]) for k in range(9)]
        ends = [int(ib[k]) for k in range(10)]
        for k in range(10):
            m[starts[k] : ends[k], k] = 1.0 / (ends[k] - starts[k])
        m[int(ib[9]) : int(ib[10]), 10] = 1.0 / (int(ib[10]) - int(ib[9]))
        m[int(ib[10]) : int(ib[11]), 11] = 1.0 / (int(ib[11]) - int(ib[10]))
        m[1 : int(ib[9]), 12] = 1.0 / (int(ib[9]) - 1)
        maskt = np.ascontiguousarray(m.reshape(T, P, NK).transpose(1, 0, 2))

        in_maps.append(
            dict(
                hidden=np.ascontiguousarray(hidden[b]),
                maskt=maskt,
                wp=wp_t,
                wa0=wa0_t,
                wa=wa_t,
                wl0=wl0_t,
                wl=wl_t,
                biases=biases,
            )
        )
    return in_maps


def _run(in_maps, **kwargs):
    return run_bass_kernel_spmd(_get_nc(), in_maps, core_ids=list(range(B)), **kwargs)


def kernel(**inputs):
    in_maps = _prep_inputs(**inputs)
    res = _run(in_maps)
    return np.stack([r["out"].reshape(O, 1) for r in res.results])


def _install_ntff_hook():
    """The RL container's antenv lacks axon_hooks, so boot() skipped NTFF
    hook registration. Recreate the module and register the ctypes hook."""
    import sys
    import types

    name = "antenv.axon_hooks"
    if name not in sys.modules:
        try:
            __import__(name)
        except ImportError:
            mod = types.ModuleType(name)
            mod._hook = None
            mod.set_axon_ntff_profile_hook = lambda h: setattr(mod, "_hook", h)
            mod.get_axon_ntff_profile_hook = lambda: mod._hook
            sys.modules[name] = mod
            import antenv

            antenv.axon_hooks = mod
    import antenv.axon_hooks as ah

    if ah.get_axon_ntff_profile_hook() is None:
        from trn_agent_boot.trn_boot import _ntff_profile_via_ctypes

        ah.set_axon_ntff_profile_hook(
            _ntff_profile_via_ctypes("/opt/axon/libaxon_pjrt.so")
        )

    # keep profile artifacts local — no fish/S3 share in this container
    import concourse.bass_utils as bu

    bu.upload_artifacts = lambda tmpdir: tmpdir


def benchmark(trace_cores=None, **inputs):
    """Run with NTFF tracing; returns (output, BassKernelResults)."""
    _install_ntff_hook()
    in_maps = _prep_inputs(**inputs)
    res = _run(in_maps, trace=True, trace_cores=trace_cores)
    out = np.stack([r["out"].reshape(O, 1) for r in res.results])
    return out, res


# revision 12
# speedup vs baseline: 2.1863x; 2.1863x over previous
"""Trainium2 Bass kernel for nn_Beta_LR_41308995453190.

Network (per (b, o) pair):
  - 13 segment means over the L axis of hidden[b, o] (ragged boundaries
    from idx[b]): 10 context segments, question, option, whole-context.
  - beta-param projection e = 1 + relu(x @ Wp + bp), split a/b.
  - three attention pools (intersection over segments, renew over
    (segment, intersection) pairs, union over inverted renewed params).
  - classify head: concat 8 beta embeddings -> relu(@Wl0 + bl0) -> @Wl + bl.

Sharding: data-parallel over the batch dim B=8 (one batch per NeuronCore),
weights replicated.

Implementation notes (the kernel is PE *instruction-issue* bound, so the
design minimizes tensor-engine instructions):
  - Segment sums are 0/1-mask matmuls (mask as the 13-column stationary
    operand, hidden streaming 512 wide), scaled by 1/count afterwards.
    Hidden and mask travel in bf16 (the mask is exactly representable);
    sums accumulate in fp32 PSUM.
  - All layer matmuls run "flipped": the small activation block is the
    stationary operand, the weight matrix streams 512 columns at a time.
    Layer outputs come out row-major and are transposed back to
    feature-major with tensor-engine transposes so the segment softmaxes
    stay free-axis reductions.
  - Wp/Wa0/Wa are bf16 (measured end-to-end error contribution 1e-6 for
    Wa0/Wa, 2e-4 for Wp); the classify head Wl0 stays fp32 (bf16 there
    would cost 2.3e-3). The whole softmax/pooling pipeline is fp32.
"""

import numpy as np
import ml_dtypes

try:
    import concourse.bass as bass
except ImportError:
    import sys

    sys.path.insert(0, "/opt/trn_rl_repo")
    import concourse.bass as bass

import concourse.tile as tile
from concourse import mybir
from concourse.bass_utils import run_bass_kernel_spmd
from concourse.masks import make_identity

F32 = mybir.dt.float32
BF16 = mybir.dt.bfloat16
NPBF16 = ml_dtypes.bfloat16
AX = mybir.AxisListType.X
OP = mybir.AluOpType
AF = mybir.ActivationFunctionType

B, O, L, E = 8, 4, 1024, 1024
BETA = 512
NSEG = 12
NK = 13  # 10 ctx + q + o + allc
P = 128
T = L // P  # 8 L-tiles per option
NCOL = O * NK  # 52


# ---------------------------------------------------------------------------
# Workaround: this neuronxcc walrus build rejects more than one sem wait per
# TPB instruction ("Too many sync wait commands"). Hoist excess waits onto
# drain instructions inserted immediately before the offending instruction on
# the same engine — the engine blocks at each drain until its condition
# holds, which is semantically identical to multiple waits on one
# instruction.
# ---------------------------------------------------------------------------
def _split_excess_waits(nc, max_waits=1):
    scratch_bb = nc.cur_bb.bb
    for f in nc.m.functions:
        for bb in f.blocks:
            new_list = []
            for ins in bb.instructions:
                si = ins.sync_info
                waits = list(si.on_wait) if si and si.on_wait else []
                if len(waits) > max_waits:
                    for w in waits[: len(waits) - max_waits]:
                        carrier = nc.engines[ins.engine].drain(fusable=False).ins
                        scratch_bb.instructions.remove(carrier)
                        carrier.sync_info = mybir.SyncInfo(
                            on_wait=[w], on_update=[]
                        )
                        new_list.append(carrier)
                    si.on_wait = waits[len(waits) - max_waits :]
                new_list.append(ins)
            bb.instructions[:] = new_list


def _build_nc(debug=False):
    nc = bass.Bass("TRN2", target_bir_lowering=False)

    hid_d = nc.dram_tensor("hidden", [O, L, E], BF16, kind="ExternalInput")
    mask_d = nc.dram_tensor("maskt", [P, T, NK], BF16, kind="ExternalInput")
    cnt_d = nc.dram_tensor("cntinv", [NK, 1], F32, kind="ExternalInput")
    wp_d = nc.dram_tensor("wp", [P, 8, 1024], BF16, kind="ExternalInput")
    wa0_d = nc.dram_tensor("wa0", [P, 8, 512], BF16, kind="ExternalInput")
    wa_d = nc.dram_tensor("wa", [P, 4, 512], BF16, kind="ExternalInput")
    wl0_d = nc.dram_tensor("wl0", [P, 32, 512], F32, kind="ExternalInput")
    bias_d = nc.dram_tensor("biases", [P, 21], F32, kind="ExternalInput")
    bl0r_d = nc.dram_tensor("bl0rep", [O, 512], F32, kind="ExternalInput")
    wlr_d = nc.dram_tensor("wlrep", [O, 512], F32, kind="ExternalInput")
    out_d = nc.dram_tensor("out", [O, 1], F32, kind="ExternalOutput")

    with tile.TileContext(nc) as tc:
        with (
            tc.tile_pool(name="const", bufs=1) as const,
            tc.tile_pool(name="hidp2", bufs=2) as hidp2,
            tc.tile_pool(name="act", bufs=1) as act,
            tc.tile_pool(name="tmp", bufs=3) as tmp,
            tc.tile_pool(name="rows", bufs=2) as rowsp,
            tc.tile_pool(name="pseg", bufs=1, space="PSUM") as pseg,
            tc.tile_pool(name="pxt", bufs=2, space="PSUM") as pxt,
            tc.tile_pool(name="prow", bufs=2, space="PSUM") as prow,
            tc.tile_pool(name="pt", bufs=2, space="PSUM") as pt,
        ):
            # ---- constants (seg-phase ones first)
            mask_sb = const.tile([P, T, NK], BF16)
            nc.sync.dma_start(out=mask_sb, in_=mask_d[:])
            cnt_sb = const.tile([NK, 1], F32)
            nc.sync.dma_start(out=cnt_sb, in_=cnt_d[:])
            bias_sb = const.tile([P, 21], F32)
            nc.sync.dma_start(out=bias_sb, in_=bias_d[:])
            bl0r_sb = const.tile([O, 512], F32)
            nc.sync.dma_start(out=bl0r_sb, in_=bl0r_d[:])
            wlr_sb = const.tile([O, 512], F32)
            nc.sync.dma_start(out=wlr_sb, in_=wlr_d[:])
            ident = const.tile([P, P], F32)
            make_identity(nc, ident)
            wp_sb = const.tile([P, 8, 1024], BF16)
            nc.sync.dma_start(out=wp_sb, in_=wp_d[:])
            wa0_sb = const.tile([P, 8, 512], BF16)
            nc.sync.dma_start(out=wa0_sb, in_=wa0_d[:])
            wa_sb = const.tile([P, 4, 512], BF16)
            nc.sync.dma_start(out=wa_sb, in_=wa_d[:])

            def bcol(i):
                return bias_sb[:, i : i + 1]

            # ---- segment sums: ps[k, e] = sum over rows of seg k (0/1 mask)
            # then x = ps * cntinv, transposed to xT[c, o, k] (bf16)
            xT = act.tile([P, 8, O, NK], BF16)
            for o in range(O):
                htiles = []
                for t in range(T):
                    htile = hidp2.tile([P, E], BF16, tag=f"htile{t}")
                    nc.sync.dma_start(
                        out=htile, in_=hid_d[o, t * P : (t + 1) * P, :]
                    )
                    htiles.append(htile)
                ps = pseg.tile([NK, E], F32, tag="ps_seg")
                for half in range(2):
                    sl = slice(half * 512, half * 512 + 512)
                    for t in range(T):
                        nc.tensor.matmul(
                            out=ps[:, sl],
                            lhsT=mask_sb[:, t, :],
                            rhs=htiles[t][:, sl],
                            start=(t == 0),
                            stop=(t == T - 1),
                        )
                x_sb = rowsp.tile([NK, E], F32, tag="x_sb")
                nc.vector.tensor_scalar_mul(
                    out=x_sb, in0=ps[:, :], scalar1=cnt_sb[:, :]
                )
                pxt_o = pxt.tile([P, 8, NK], F32, tag="pxt")
                for c in range(8):
                    nc.tensor.transpose(
                        out=pxt_o[:, c, :],
                        in_=x_sb[:, c * P : (c + 1) * P],
                        identity=ident[:NK, :NK],
                    )
                nc.scalar.copy(out=xT[:, :, o, :], in_=pxt_o[:, :, :])

            # ---- wl0 DMA last: only needed by the classify head
            wl0_sb = const.tile([P, 32, 512], F32)
            nc.sync.dma_start(out=wl0_sb, in_=wl0_d[:])

            def flip_layer(
                name,
                lhs_chunks,  # list of bf16 [P, R] stationary APs (K chunks)
                w_sb,  # weight tile, [P, K/128, NW] layout
                n_out,  # output features
                r,  # rows (= lhs free size)
            ):
                """out rows = (lhs^T)^T @ W, returns list of fp32 PSUM tiles
                [r, 512] per 512-wide output chunk, and the row-major sbuf
                copy [r, n_out]."""
                rows_sb = rowsp.tile([r, n_out], F32, tag=f"rows_{name}")
                psums = []
                for n2 in range(n_out // 512):
                    pr = prow.tile([r, 512], F32, tag="prow")
                    for c, lhs in enumerate(lhs_chunks):
                        nc.tensor.matmul(
                            out=pr,
                            lhsT=lhs,
                            rhs=w_sb[:, c, n2 * 512 : (n2 + 1) * 512]
                            if w_sb.shape[2] > 512
                            else w_sb[:, c, :],
                            start=(c == 0),
                            stop=(c == len(lhs_chunks) - 1),
                        )
                    nc.scalar.copy(
                        out=rows_sb[:, n2 * 512 : (n2 + 1) * 512], in_=pr[:, :]
                    )
                    psums.append(pr)
                return rows_sb

            def transpose_rows(rows_sb, r, n_out):
                """Yield (mc, psum [P, r]) transposed feature chunks."""
                for mc in range(n_out // P):
                    ptile = pt.tile([P, r], F32, tag="pt")
                    nc.tensor.transpose(
                        out=ptile,
                        in_=rows_sb[:, mc * P : (mc + 1) * P],
                        identity=ident[:r, :r],
                    )
                    yield mc, ptile

            # ---- projection: e = max(x @ Wp + (bp + 1), 1)
            eT = act.tile([P, 8, O, NK], F32)
            eTb = act.tile([P, 8, NCOL], BF16)
            xT_chunks = [xT[:, c, :, :] for c in range(8)]
            rows_e = flip_layer("e", xT_chunks, wp_sb, 1024, NCOL)
            for mc, ptile in transpose_rows(rows_e, NCOL, 1024):
                nc.vector.tensor_scalar(
                    out=eT[:, mc, :, :],
                    in0=ptile[:, :],
                    scalar1=bcol(mc),
                    scalar2=1.0,
                    op0=OP.add,
                    op1=OP.max,
                )
                nc.vector.tensor_copy(out=eTb[:, mc, :], in_=eT[:, mc, :, :])

            # ---- pool 1 (intersection): h1 = relu(e @ Wa0 + ba0) (bf16 out)
            h1Tb = act.tile([P, 4, NCOL], BF16)
            rows_h1 = flip_layer(
                "h1", [eTb[:, c, :] for c in range(8)], wa0_sb, 512, NCOL
            )
            for mc, ptile in transpose_rows(rows_h1, NCOL, 512):
                nc.vector.tensor_scalar(
                    out=h1Tb[:, mc, :],
                    in0=ptile[:, :],
                    scalar1=bcol(8 + mc),
                    scalar2=0.0,
                    op0=OP.add,
                    op1=OP.max,
                )

            # l1 = h1 @ Wa + ba (fp32, shared by pool 1 softmax and renew)
            l1T = act.tile([P, 4, O, NK], F32)
            rows_l1 = flip_layer(
                "l1", [h1Tb[:, c, :] for c in range(4)], wa_sb, 512, NCOL
            )
            for mc, ptile in transpose_rows(rows_l1, NCOL, 512):
                nc.vector.tensor_scalar_add(
                    out=l1T[:, mc, :, :], in0=ptile[:, :], scalar1=bcol(12 + mc)
                )

            # pool 1 softmax over the 10 ctx segments + weighted reduce
            cat2 = act.tile([P, 8, O], F32)
            cat2b = act.tile([P, 8, O], BF16)
            for m in range(4):
                lsl = l1T[:, m, :, 0:10]
                mx = tmp.tile([P, O], F32, tag="mx")
                nc.vector.reduce_max(mx, lsl, axis=AX)
                d = tmp.tile([P, O, 10], F32, tag="d")
                nc.vector.tensor_tensor(
                    out=d, in0=lsl, in1=mx.broadcast_to([P, O, 10]), op=OP.subtract
                )
                w = tmp.tile([P, O, 10], F32, tag="w")
                nc.scalar.activation(out=w, in_=d, func=AF.Exp)
                s = tmp.tile([P, O], F32, tag="s")
                nc.vector.reduce_sum(s, w, axis=AX)
                r = tmp.tile([P, O], F32, tag="r")
                nc.vector.reciprocal(out=r, in_=s)
                wn = tmp.tile([P, O, 10], F32, tag="wn")
                nc.vector.tensor_tensor(
                    out=wn, in0=w, in1=r.broadcast_to([P, O, 10]), op=OP.mult
                )
                wa_t = tmp.tile([P, O, 10], F32, tag="wa_t")
                nc.vector.tensor_tensor(
                    out=wa_t, in0=wn, in1=eT[:, m, :, 0:10], op=OP.mult
                )
                nc.vector.reduce_sum(cat2[:, m, :], wa_t, axis=AX)
                wb_t = tmp.tile([P, O, 10], F32, tag="wb_t")
                nc.vector.tensor_tensor(
                    out=wb_t, in0=wn, in1=eT[:, 4 + m, :, 0:10], op=OP.mult
                )
                nc.vector.reduce_sum(cat2[:, 4 + m, :], wb_t, axis=AX)
            nc.vector.tensor_copy(out=cat2b, in_=cat2)

            # ---- renew: h2/l2 for the intersection pair element
            h2Tb = act.tile([P, 4, O], BF16)
            rows_h2 = flip_layer(
                "h2", [cat2b[:, c, :] for c in range(8)], wa0_sb, 512, O
            )
            for mc, ptile in transpose_rows(rows_h2, O, 512):
                nc.vector.tensor_scalar(
                    out=h2Tb[:, mc, :],
                    in0=ptile[:, :],
                    scalar1=bcol(8 + mc),
                    scalar2=0.0,
                    op0=OP.add,
                    op1=OP.max,
                )
            l2T = act.tile([P, 4, O], F32)
            rows_l2 = flip_layer(
                "l2", [h2Tb[:, c, :] for c in range(4)], wa_sb, 512, O
            )
            for mc, ptile in transpose_rows(rows_l2, O, 512):
                nc.vector.tensor_scalar_add(
                    out=l2T[:, mc, :], in0=ptile[:, :], scalar1=bcol(12 + mc)
                )

            # pair softmax([l1[k], l2]) -> na/nb; store reciprocals
            raT = act.tile([P, 4, O, 10], F32)
            rbT = act.tile([P, 4, O, 10], F32)
            raTb = act.tile([P, 4, O, 10], BF16)
            rbTb = act.tile([P, 4, O, 10], BF16)
            for m in range(4):
                l1s = l1T[:, m, :, 0:10]
                l2b = l2T[:, m, :].broadcast_to([P, O, 10])
                mxp = tmp.tile([P, O, 10], F32, tag="mxp")
                nc.vector.tensor_tensor(out=mxp, in0=l1s, in1=l2b, op=OP.max)
                d1 = tmp.tile([P, O, 10], F32, tag="d1")
                nc.vector.tensor_tensor(out=d1, in0=l1s, in1=mxp, op=OP.subtract)
                e1 = tmp.tile([P, O, 10], F32, tag="e1")
                nc.scalar.activation(out=e1, in_=d1, func=AF.Exp)
                d2 = tmp.tile([P, O, 10], F32, tag="d2")
                nc.vector.tensor_tensor(out=d2, in0=l2b, in1=mxp, op=OP.subtract)
                e2 = tmp.tile([P, O, 10], F32, tag="e2")
                nc.scalar.activation(out=e2, in_=d2, func=AF.Exp)
                s12 = tmp.tile([P, O, 10], F32, tag="s12")
                nc.vector.tensor_tensor(out=s12, in0=e1, in1=e2, op=OP.add)
                rs = tmp.tile([P, O, 10], F32, tag="rs")
                nc.vector.reciprocal(out=rs, in_=s12)
                for half, dst, dstb in ((0, raT, raTb), (1, rbT, rbTb)):
                    t1 = tmp.tile([P, O, 10], F32, tag="t1")
                    nc.vector.tensor_tensor(
                        out=t1, in0=e1, in1=eT[:, half * 4 + m, :, 0:10], op=OP.mult
                    )
                    t2 = tmp.tile([P, O, 10], F32, tag="t2")
                    nc.vector.tensor_tensor(
                        out=t2,
                        in0=e2,
                        in1=cat2[:, half * 4 + m, :].broadcast_to([P, O, 10]),
                        op=OP.mult,
                    )
                    t3 = tmp.tile([P, O, 10], F32, tag="t3")
                    nc.vector.tensor_tensor(out=t3, in0=t1, in1=t2, op=OP.add)
                    nv = tmp.tile([P, O, 10], F32, tag="nv")
                    nc.vector.tensor_tensor(out=nv, in0=t3, in1=rs, op=OP.mult)
                    nc.vector.reciprocal(out=dst[:, m, :, :], in_=nv)
                    nc.vector.tensor_copy(
                        out=dstb[:, m, :, :], in_=dst[:, m, :, :]
                    )

            # ---- union pool over segments of [1/na; 1/nb]
            h3Tb = act.tile([P, 4, O, 10], BF16)
            rows_h3 = flip_layer(
                "h3",
                [raTb[:, c, :, :] for c in range(4)]
                + [rbTb[:, c, :, :] for c in range(4)],
                wa0_sb,
                512,
                O * 10,
            )
            for mc, ptile in transpose_rows(rows_h3, O * 10, 512):
                nc.vector.tensor_scalar(
                    out=h3Tb[:, mc, :, :],
                    in0=ptile[:, :],
                    scalar1=bcol(8 + mc),
                    scalar2=0.0,
                    op0=OP.add,
                    op1=OP.max,
                )
            l3T = act.tile([P, 4, O, 10], F32)
            rows_l3 = flip_layer(
                "l3", [h3Tb[:, c, :, :] for c in range(4)], wa_sb, 512, O * 10
            )
            for mc, ptile in transpose_rows(rows_l3, O * 10, 512):
                nc.vector.tensor_scalar_add(
                    out=l3T[:, mc, :, :], in0=ptile[:, :], scalar1=bcol(12 + mc)
                )

            # union softmax + weighted reduce + invert -> catF chunks 0..7
            catF = act.tile([P, 32, O], F32)
            for m in range(4):
                l3s = l3T[:, m, :, :]
                mx3 = tmp.tile([P, O], F32, tag="mx3")
                nc.vector.reduce_max(mx3, l3s, axis=AX)
                d3 = tmp.tile([P, O, 10], F32, tag="d3")
                nc.vector.tensor_tensor(
                    out=d3, in0=l3s, in1=mx3.broadcast_to([P, O, 10]), op=OP.subtract
                )
                w3 = tmp.tile([P, O, 10], F32, tag="w3")
                nc.scalar.activation(out=w3, in_=d3, func=AF.Exp)
                s3 = tmp.tile([P, O], F32, tag="s3")
                nc.vector.reduce_sum(s3, w3, axis=AX)
                r3 = tmp.tile([P, O], F32, tag="r3")
                nc.vector.reciprocal(out=r3, in_=s3)
                wn3 = tmp.tile([P, O, 10], F32, tag="wn3")
                nc.vector.tensor_tensor(
                    out=wn3, in0=w3, in1=r3.broadcast_to([P, O, 10]), op=OP.mult
                )
                for half, src in ((0, raT), (1, rbT)):
                    tu = tmp.tile([P, O, 10], F32, tag="tu")
                    nc.vector.tensor_tensor(
                        out=tu, in0=wn3, in1=src[:, m, :, :], op=OP.mult
                    )
                    su = tmp.tile([P, O], F32, tag="su")
                    nc.vector.reduce_sum(su, tu, axis=AX)
                    nc.vector.reciprocal(out=catF[:, half * 4 + m, :], in_=su)

            # catF chunks 8..31: a_ac, b_ac, a_o, b_o, a_q, b_q from eT
            for j, (half, k) in enumerate(
                ((0, 12), (1, 12), (0, 11), (1, 11), (0, 10), (1, 10))
            ):
                nc.gpsimd.tensor_copy(
                    out=catF[:, 8 + j * 4 : 12 + j * 4, :],
                    in_=eT[:, half * 4 : half * 4 + 4, :, k],
                )

            # ---- classify head (fp32): hf = cat @ Wl0, rows [O, 512]
            pf = prow.tile([O, 512], F32, tag="prow")
            for kc in range(32):
                nc.tensor.matmul(
                    out=pf,
                    lhsT=catF[:, kc, :],
                    rhs=wl0_sb[:, kc, :],
                    start=(kc == 0),
                    stop=(kc == 31),
                )
            # out = relu(hf + bl0) . Wl + bl, all on the vector engine
            hrelu = rowsp.tile([O, 512], F32, tag="hrelu")
            nc.vector.tensor_tensor(out=hrelu, in0=pf[:, :], in1=bl0r_sb, op=OP.add)
            nc.vector.tensor_scalar_max(out=hrelu, in0=hrelu, scalar1=0.0)
            hw = rowsp.tile([O, 512], F32, tag="hw")
            nc.vector.tensor_tensor(out=hw, in0=hrelu, in1=wlr_sb, op=OP.mult)
            osum = rowsp.tile([O, 1], F32, tag="osum")
            nc.vector.reduce_sum(osum, hw, axis=AX)
            out_sb = rowsp.tile([O, 1], F32, tag="out_sb")
            nc.vector.tensor_scalar_add(
                out=out_sb, in0=osum, scalar1=bias_sb[0:O, 20:21]
            )
            nc.sync.dma_start(out=out_d[:], in_=out_sb)

            if debug:
                for name, t in (
                    ("xT", xT),
                    ("eT", eT),
                    ("l1T", l1T),
                    ("cat2", cat2),
                    ("raT", raT),
                    ("rbT", rbT),
                    ("catF", catF),
                ):
                    dt = F32 if t is not xT else BF16
                    d = nc.dram_tensor(
                        "dbg_" + name, list(t.shape), dt, kind="ExternalOutput"
                    )
                    nc.sync.dma_start(out=d[:], in_=t)

    _split_excess_waits(nc)
    return nc


_NC = None


def _get_nc():
    global _NC
    if _NC is None:
        _NC = _build_nc()
    return _NC


def _prep_inputs(hidden, idx, Wp, bp, Wa0, ba0, Wa, ba, Wl0, bl0, Wl, bl):
    hidden = np.asarray(hidden, dtype=np.float32)
    idx = np.asarray(idx).astype(np.int64)

    f32 = lambda a: np.ascontiguousarray(np.asarray(a, dtype=np.float32))
    bf = lambda a: np.ascontiguousarray(np.asarray(a, dtype=np.float32).astype(NPBF16))
    bp, ba0, ba, bl0, bl = f32(bp), f32(ba0), f32(ba), f32(bl0), f32(bl)
    Wl = f32(Wl)

    hid_b = np.ascontiguousarray(hidden.astype(NPBF16))  # [B, O, L, E]
    wp_t = bf(np.asarray(Wp, np.float32).reshape(8, P, 1024).transpose(1, 0, 2))
    wa0_t = bf(np.asarray(Wa0, np.float32).reshape(8, P, 512).transpose(1, 0, 2))
    wa_t = bf(np.asarray(Wa, np.float32).reshape(4, P, 512).transpose(1, 0, 2))
    wl0_t = f32(
        np.asarray(Wl0, np.float32).reshape(32, P, 512).transpose(1, 0, 2)
    )

    biases = np.zeros((P, 21), dtype=np.float32)
    biases[:, 0:8] = (bp + 1.0).reshape(8, P).T
    biases[:, 8:12] = ba0.reshape(4, P).T
    biases[:, 12:16] = ba.reshape(4, P).T
    biases[:, 16:20] = bl0.reshape(4, P).T
    biases[:, 20] = bl[0]

    bl0rep = np.ascontiguousarray(np.broadcast_to(bl0, (O, 512)).astype(np.float32))
    wlrep = np.ascontiguousarray(np.broadcast_to(Wl[:, 0], (O, 512)).astype(np.float32))

    in_maps = []
    for b in range(B):
        m = np.zeros((L, NK), dtype=np.float32)
        cntinv = np.zeros((NK, 1), dtype=np.float32)
        ib = idx[b]
        starts = [1] + [int(ib[k]) for k in range(9)]
        ends = [int(ib[k]) for k in range(10)]
        segs = [(starts[k], ends[k]) for k in range(10)]
        segs.append((int(ib[9]), int(ib[10])))
        segs.append((int(ib[10]), int(ib[11])))
        segs.append((1, int(ib[9])))
        for k, (s, e) in enumerate(segs):
            m[s:e, k] = 1.0
            cntinv[k, 0] = 1.0 / (e - s)
        maskt = np.ascontiguousarray(
            m.reshape(T, P, NK).transpose(1, 0, 2).astype(NPBF16)
        )

        in_maps.append(
            dict(
                hidden=np.ascontiguousarray(hid_b[b]),
                maskt=maskt,
                cntinv=cntinv,
                wp=wp_t,
                wa0=wa0_t,
                wa=wa_t,
                wl0=wl0_t,
                biases=biases,
                bl0rep=bl0rep,
                wlrep=wlrep,
            )
        )
    return in_maps


def _run(in_maps, **kwargs):
    return run_bass_kernel_spmd(_get_nc(), in_maps, core_ids=list(range(B)), **kwargs)


def kernel(**inputs):
    in_maps = _prep_inputs(**inputs)
    res = _run(in_maps)
    return np.stack([r["out"].reshape(O, 1) for r in res.results])


def _install_ntff_hook():
    """The RL container's antenv lacks axon_hooks, so boot() skipped NTFF
    hook registration. Recreate the module and register the ctypes hook."""
    import sys
    import types

    name = "antenv.axon_hooks"
    if name not in sys.modules:
        try:
            __import__(name)
        except ImportError:
            mod = types.ModuleType(name)
            mod._hook = None
            mod.set_axon_ntff_profile_hook = lambda h: setattr(mod, "_hook", h)
            mod.get_axon_ntff_profile_hook = lambda: mod._hook
            sys.modules[name] = mod
            import antenv

            antenv.axon_hooks = mod
    import antenv.axon_hooks as ah

    if ah.get_axon_ntff_profile_hook() is None:
        from trn_agent_boot.trn_boot import _ntff_profile_via_ctypes

        ah.set_axon_ntff_profile_hook(
            _ntff_profile_via_ctypes("/opt/axon/libaxon_pjrt.so")
        )

    import concourse.bass_utils as bu

    bu.upload_artifacts = lambda tmpdir: tmpdir


def benchmark(trace_cores=None, **inputs):
    """Run with NTFF tracing; returns (output, BassKernelResults)."""
    _install_ntff_hook()
    in_maps = _prep_inputs(**inputs)
    res = _run(in_maps, trace=True, trace_cores=trace_cores)
    out = np.stack([r["out"].reshape(O, 1) for r in res.results])
    return out, res


# revision 19
# speedup vs baseline: 2.2607x; 1.0340x over previous
"""Trainium2 Bass kernel for nn_Beta_LR_41308995453190.

Network (per (b, o) pair):
  - 13 segment means over the L axis of hidden[b, o] (ragged boundaries
    from idx[b]): 10 context segments, question, option, whole-context.
  - beta-param projection e = 1 + relu(x @ Wp + bp), split a/b.
  - three attention pools (intersection over segments, renew over
    (segment, intersection) pairs, union over inverted renewed params).
  - classify head: concat 8 beta embeddings -> relu(@Wl0 + bl0) -> @Wl + bl.

Sharding: data-parallel over the batch dim B=8 (one batch per NeuronCore),
weights replicated.

Implementation notes (the kernel is PE *instruction-issue* bound, so the
design minimizes tensor-engine instructions):
  - Segment sums are 0/1-mask matmuls (mask as the 13-column stationary
    operand, hidden streaming 512 wide), scaled by 1/count afterwards.
    Hidden and mask travel in bf16 (the mask is exactly representable);
    sums accumulate in fp32 PSUM.
  - All layer matmuls run "flipped": the small activation block is the
    stationary operand, the weight matrix streams 512 columns at a time.
    Layer outputs come out row-major and are transposed back to
    feature-major with tensor-engine transposes so the segment softmaxes
    stay free-axis reductions.
  - Wp/Wa0/Wa are bf16 (measured end-to-end error contribution 1e-6 for
    Wa0/Wa, 2e-4 for Wp); the classify head Wl0 stays fp32 (bf16 there
    would cost 2.3e-3). The whole softmax/pooling pipeline is fp32.
"""

import numpy as np
import ml_dtypes

try:
    import concourse.bass as bass
except ImportError:
    import sys

    sys.path.insert(0, "/opt/trn_rl_repo")
    import concourse.bass as bass

import concourse.tile as tile
from concourse import mybir
from concourse.bass_utils import run_bass_kernel_spmd
from concourse.masks import make_identity

F32 = mybir.dt.float32
BF16 = mybir.dt.bfloat16
NPBF16 = ml_dtypes.bfloat16
AX = mybir.AxisListType.X
OP = mybir.AluOpType
AF = mybir.ActivationFunctionType

B, O, L, E = 8, 4, 1024, 1024
BETA = 512
NSEG = 12
NK = 13  # 10 ctx + q + o + allc
P = 128
T = L // P  # 8 L-tiles per option
NCOL = O * NK  # 52


# ---------------------------------------------------------------------------
# Workaround: this neuronxcc walrus build rejects more than one sem wait per
# TPB instruction ("Too many sync wait commands"). Hoist excess waits onto
# drain instructions inserted immediately before the offending instruction on
# the same engine — the engine blocks at each drain until its condition
# holds, which is semantically identical to multiple waits on one
# instruction.
# ---------------------------------------------------------------------------
def _split_excess_waits(nc, max_waits=1):
    scratch_bb = nc.cur_bb.bb
    for f in nc.m.functions:
        for bb in f.blocks:
            new_list = []
            for ins in bb.instructions:
                si = ins.sync_info
                waits = list(si.on_wait) if si and si.on_wait else []
                if len(waits) > max_waits:
                    for w in waits[: len(waits) - max_waits]:
                        carrier = nc.engines[ins.engine].nop(nofuse=True).ins
                        scratch_bb.instructions.remove(carrier)
                        carrier.sync_info = mybir.SyncInfo(
                            on_wait=[w], on_update=[]
                        )
                        new_list.append(carrier)
                    si.on_wait = waits[len(waits) - max_waits :]
                new_list.append(ins)
            bb.instructions[:] = new_list


def _build_nc(debug=False):
    nc = bass.Bass("TRN2", target_bir_lowering=False)

    hid_d = nc.dram_tensor("hidden", [O, L, E], BF16, kind="ExternalInput")
    mask_d = nc.dram_tensor("maskt", [P, T, NK], BF16, kind="ExternalInput")
    cnt_d = nc.dram_tensor("cntinv", [NK, 1], F32, kind="ExternalInput")
    wp_d = nc.dram_tensor("wp", [P, 8, 1024], BF16, kind="ExternalInput")
    wa0_d = nc.dram_tensor("wa0", [P, 8, 512], BF16, kind="ExternalInput")
    wa_d = nc.dram_tensor("wa", [P, 4, 512], BF16, kind="ExternalInput")
    wl0_d = nc.dram_tensor("wl0", [P, 32, 512], F32, kind="ExternalInput")
    bias_d = nc.dram_tensor("biases", [P, 21], F32, kind="ExternalInput")
    bl0r_d = nc.dram_tensor("bl0rep", [O, 512], F32, kind="ExternalInput")
    wlr_d = nc.dram_tensor("wlrep", [O, 512], F32, kind="ExternalInput")
    out_d = nc.dram_tensor("out", [O, 1], F32, kind="ExternalOutput")

    with tile.TileContext(nc) as tc:
        with (
            tc.tile_pool(name="const", bufs=1) as const,
            tc.tile_pool(name="hidp2", bufs=2) as hidp2,
            tc.tile_pool(name="act", bufs=1) as act,
            tc.tile_pool(name="tmp", bufs=2) as tmp,
            tc.tile_pool(name="rows", bufs=1) as rowsp,
            tc.tile_pool(name="pseg", bufs=1, space="PSUM") as pseg,
            tc.tile_pool(name="pxt", bufs=2, space="PSUM") as pxt,
            tc.tile_pool(name="prow", bufs=2, space="PSUM") as prow,
            tc.tile_pool(name="pt", bufs=2, space="PSUM") as pt,
        ):
            # ---- constants (seg-phase ones first)
            mask_sb = const.tile([P, T, NK], BF16)
            nc.sync.dma_start(out=mask_sb, in_=mask_d[:])
            cnt_sb = const.tile([NK, 1], F32)
            nc.sync.dma_start(out=cnt_sb, in_=cnt_d[:])
            bias_sb = const.tile([P, 21], F32)
            nc.sync.dma_start(out=bias_sb, in_=bias_d[:])
            bl0r_sb = const.tile([O, 512], F32)
            nc.sync.dma_start(out=bl0r_sb, in_=bl0r_d[:])
            wlr_sb = const.tile([O, 512], F32)
            nc.sync.dma_start(out=wlr_sb, in_=wlr_d[:])
            ident = const.tile([P, P], F32)
            make_identity(nc, ident)

            def bcol(i):
                return bias_sb[:, i : i + 1]

            # ---- segment sums: ps[k, e] = sum over rows of seg k (0/1 mask)
            # then x = ps * cntinv, transposed to xT[c, o, k] (bf16)
            xT = act.tile([P, 8, O, NK], BF16)
            wp_sb = wa0_sb = wa_sb = None
            for o in range(O):
                htiles = []
                for t in range(T):
                    htile = hidp2.tile([P, E], BF16, tag=f"htile{t}")
                    nc.sync.dma_start(
                        out=htile, in_=hid_d[o, t * P : (t + 1) * P, :]
                    )
                    htiles.append(htile)
                if o == 0:
                    # queue the head weights behind the first option's tiles
                    wp_sb = const.tile([P, 8, 1024], BF16)
                    nc.sync.dma_start(out=wp_sb, in_=wp_d[:])
                    wa0_sb = const.tile([P, 8, 512], BF16)
                    nc.sync.dma_start(out=wa0_sb, in_=wa0_d[:])
                    wa_sb = const.tile([P, 4, 512], BF16)
                    nc.sync.dma_start(out=wa_sb, in_=wa_d[:])
                ps = pseg.tile([NK, E], F32, tag="ps_seg")
                for half in range(2):
                    sl = slice(half * 512, half * 512 + 512)
                    for t in range(T):
                        nc.tensor.matmul(
                            out=ps[:, sl],
                            lhsT=mask_sb[:, t, :],
                            rhs=htiles[t][:, sl],
                            start=(t == 0),
                            stop=(t == T - 1),
                        )
                x_sb = rowsp.tile([NK, E], F32, tag="x_sb")
                nc.vector.tensor_scalar_mul(
                    out=x_sb, in0=ps[:, :], scalar1=cnt_sb[:, :]
                )
                pxt_o = pxt.tile([P, 8, NK], F32, tag="pxt")
                for c in range(8):
                    nc.tensor.transpose(
                        out=pxt_o[:, c, :],
                        in_=x_sb[:, c * P : (c + 1) * P],
                        identity=ident[:NK, :NK],
                    )
                nc.scalar.copy(out=xT[:, :, o, :], in_=pxt_o[:, :, :])

            # ---- wl0 DMA last: only needed by the classify head
            wl0_sb = const.tile([P, 32, 512], F32)
            nc.sync.dma_start(out=wl0_sb, in_=wl0_d[:])

            def flip_layer(
                name,
                lhs_chunks,  # list of bf16 [P, R] stationary APs (K chunks)
                w_sb,  # weight tile, [P, K/128, NW] layout
                n_out,  # output features
                r,  # rows (= lhs free size)
            ):
                """out rows = (lhs^T)^T @ W, returns list of fp32 PSUM tiles
                [r, 512] per 512-wide output chunk, and the row-major sbuf
                copy [r, n_out]."""
                rows_sb = rowsp.tile([r, n_out], F32, tag=f"rows_{name}")
                psums = []
                for n2 in range(n_out // 512):
                    pr = prow.tile([r, 512], F32, tag="prow")
                    for c, lhs in enumerate(lhs_chunks):
                        nc.tensor.matmul(
                            out=pr,
                            lhsT=lhs,
                            rhs=w_sb[:, c, n2 * 512 : (n2 + 1) * 512]
                            if w_sb.shape[2] > 512
                            else w_sb[:, c, :],
                            start=(c == 0),
                            stop=(c == len(lhs_chunks) - 1),
                        )
                    nc.scalar.copy(
                        out=rows_sb[:, n2 * 512 : (n2 + 1) * 512], in_=pr[:, :]
                    )
                    psums.append(pr)
                return rows_sb

            def transpose_rows(rows_sb, r, n_out):
                """Yield (mc, psum [P, r]) transposed feature chunks."""
                for mc in range(n_out // P):
                    ptile = pt.tile([P, r], F32, tag="pt")
                    nc.tensor.transpose(
                        out=ptile,
                        in_=rows_sb[:, mc * P : (mc + 1) * P],
                        identity=ident[:r, :r],
                    )
                    yield mc, ptile

            # ---- projection: e = max(x @ Wp + (bp + 1), 1)
            eT = act.tile([P, 8, O, NK], F32)
            eTb = act.tile([P, 8, NCOL], BF16)
            xT_chunks = [xT[:, c, :, :] for c in range(8)]
            rows_e = flip_layer("e", xT_chunks, wp_sb, 1024, NCOL)
            for mc, ptile in transpose_rows(rows_e, NCOL, 1024):
                nc.vector.tensor_scalar(
                    out=eT[:, mc, :, :],
                    in0=ptile[:, :],
                    scalar1=bcol(mc),
                    scalar2=1.0,
                    op0=OP.add,
                    op1=OP.max,
                )
                nc.vector.tensor_copy(out=eTb[:, mc, :], in_=eT[:, mc, :, :])

            # ---- pool 1 (intersection): h1 = relu(e @ Wa0 + ba0) (bf16 out)
            h1Tb = act.tile([P, 4, NCOL], BF16)
            rows_h1 = flip_layer(
                "h1", [eTb[:, c, :] for c in range(8)], wa0_sb, 512, NCOL
            )
            for mc, ptile in transpose_rows(rows_h1, NCOL, 512):
                nc.vector.tensor_scalar(
                    out=h1Tb[:, mc, :],
                    in0=ptile[:, :],
                    scalar1=bcol(8 + mc),
                    scalar2=0.0,
                    op0=OP.add,
                    op1=OP.max,
                )

            # l1 = h1 @ Wa + ba (fp32, shared by pool 1 softmax and renew)
            l1T = act.tile([P, 4, O, NK], F32)
            rows_l1 = flip_layer(
                "l1", [h1Tb[:, c, :] for c in range(4)], wa_sb, 512, NCOL
            )
            for mc, ptile in transpose_rows(rows_l1, NCOL, 512):
                nc.vector.tensor_scalar_add(
                    out=l1T[:, mc, :, :], in0=ptile[:, :], scalar1=bcol(12 + mc)
                )

            # pool 1 softmax over the 10 ctx segments + weighted reduce
            # (batched across all 4 feature chunks: [P, 4, O, 10] at once)
            cat2 = act.tile([P, 8, O], F32)
            cat2b = act.tile([P, 8, O], BF16)
            lsl = l1T[:, :, :, 0:10]
            mx = tmp.tile([P, 4, O], F32, tag="mx")
            nc.vector.reduce_max(mx, lsl, axis=AX)
            d = tmp.tile([P, 4, O, 10], F32, tag="d")
            nc.vector.tensor_tensor(
                out=d, in0=lsl, in1=mx.broadcast_to([P, 4, O, 10]), op=OP.subtract
            )
            w = tmp.tile([P, 4, O, 10], F32, tag="w")
            nc.scalar.activation(out=w, in_=d, func=AF.Exp)
            s = tmp.tile([P, 4, O], F32, tag="s")
            nc.vector.reduce_sum(s, w, axis=AX)
            r = tmp.tile([P, 4, O], F32, tag="r")
            nc.vector.reciprocal(out=r, in_=s)
            wn = tmp.tile([P, 4, O, 10], F32, tag="wn")
            nc.vector.tensor_tensor(
                out=wn, in0=w, in1=r.broadcast_to([P, 4, O, 10]), op=OP.mult
            )
            wa_t = tmp.tile([P, 4, O, 10], F32, tag="wa_t")
            nc.vector.tensor_tensor(
                out=wa_t, in0=wn, in1=eT[:, 0:4, :, 0:10], op=OP.mult
            )
            nc.vector.reduce_sum(cat2[:, 0:4, :], wa_t, axis=AX)
            wb_t = tmp.tile([P, 4, O, 10], F32, tag="wb_t")
            nc.vector.tensor_tensor(
                out=wb_t, in0=wn, in1=eT[:, 4:8, :, 0:10], op=OP.mult
            )
            nc.vector.reduce_sum(cat2[:, 4:8, :], wb_t, axis=AX)
            nc.vector.tensor_copy(out=cat2b, in_=cat2)

            # ---- renew: h2/l2 for the intersection pair element
            h2Tb = act.tile([P, 4, O], BF16)
            rows_h2 = flip_layer(
                "h2", [cat2b[:, c, :] for c in range(8)], wa0_sb, 512, O
            )
            for mc, ptile in transpose_rows(rows_h2, O, 512):
                nc.vector.tensor_scalar(
                    out=h2Tb[:, mc, :],
                    in0=ptile[:, :],
                    scalar1=bcol(8 + mc),
                    scalar2=0.0,
                    op0=OP.add,
                    op1=OP.max,
                )
            l2T = act.tile([P, 4, O], F32)
            rows_l2 = flip_layer(
                "l2", [h2Tb[:, c, :] for c in range(4)], wa_sb, 512, O
            )
            for mc, ptile in transpose_rows(rows_l2, O, 512):
                nc.vector.tensor_scalar_add(
                    out=l2T[:, mc, :], in0=ptile[:, :], scalar1=bcol(12 + mc)
                )

            # pair softmax([l1[k], l2]) -> na/nb; store reciprocals
            # (batched: [P, 4, O, 10] at once)
            raT = act.tile([P, 4, O, 10], F32)
            rbT = act.tile([P, 4, O, 10], F32)
            raTb = act.tile([P, 4, O, 10], BF16)
            rbTb = act.tile([P, 4, O, 10], BF16)
            l1s = l1T[:, :, :, 0:10]
            l2b = l2T[:, :, :].broadcast_to([P, 4, O, 10])
            mxp = tmp.tile([P, 4, O, 10], F32, tag="mxp")
            nc.vector.tensor_tensor(out=mxp, in0=l1s, in1=l2b, op=OP.max)
            d1 = tmp.tile([P, 4, O, 10], F32, tag="d1")
            nc.vector.tensor_tensor(out=d1, in0=l1s, in1=mxp, op=OP.subtract)
            e1 = tmp.tile([P, 4, O, 10], F32, tag="e1")
            nc.scalar.activation(out=e1, in_=d1, func=AF.Exp)
            d2 = tmp.tile([P, 4, O, 10], F32, tag="d2")
            nc.vector.tensor_tensor(out=d2, in0=l2b, in1=mxp, op=OP.subtract)
            e2 = tmp.tile([P, 4, O, 10], F32, tag="e2")
            nc.scalar.activation(out=e2, in_=d2, func=AF.Exp)
            s12 = tmp.tile([P, 4, O, 10], F32, tag="s12")
            nc.vector.tensor_tensor(out=s12, in0=e1, in1=e2, op=OP.add)
            rs = tmp.tile([P, 4, O, 10], F32, tag="rs")
            nc.vector.reciprocal(out=rs, in_=s12)
            for half, dst, dstb in ((0, raT, raTb), (1, rbT, rbTb)):
                t1 = tmp.tile([P, 4, O, 10], F32, tag="t1")
                nc.vector.tensor_tensor(
                    out=t1,
                    in0=e1,
                    in1=eT[:, half * 4 : half * 4 + 4, :, 0:10],
                    op=OP.mult,
                )
                t2 = tmp.tile([P, 4, O, 10], F32, tag="t2")
                nc.vector.tensor_tensor(
                    out=t2,
                    in0=e2,
                    in1=cat2[:, half * 4 : half * 4 + 4, :].broadcast_to(
                        [P, 4, O, 10]
                    ),
                    op=OP.mult,
                )
                t3 = tmp.tile([P, 4, O, 10], F32, tag="t3")
                nc.vector.tensor_tensor(out=t3, in0=t1, in1=t2, op=OP.add)
                nv = tmp.tile([P, 4, O, 10], F32, tag="nv")
                nc.vector.tensor_tensor(out=nv, in0=t3, in1=rs, op=OP.mult)
                nc.vector.reciprocal(out=dst[:, :, :, :], in_=nv)
                nc.vector.tensor_copy(out=dstb[:, :, :, :], in_=dst[:, :, :, :])

            # ---- union pool over segments of [1/na; 1/nb]
            h3Tb = act.tile([P, 4, O, 10], BF16)
            rows_h3 = flip_layer(
                "h3",
                [raTb[:, c, :, :] for c in range(4)]
                + [rbTb[:, c, :, :] for c in range(4)],
                wa0_sb,
                512,
                O * 10,
            )
            for mc, ptile in transpose_rows(rows_h3, O * 10, 512):
                nc.vector.tensor_scalar(
                    out=h3Tb[:, mc, :, :],
                    in0=ptile[:, :],
                    scalar1=bcol(8 + mc),
                    scalar2=0.0,
                    op0=OP.add,
                    op1=OP.max,
                )
            l3T = act.tile([P, 4, O, 10], F32)
            rows_l3 = flip_layer(
                "l3", [h3Tb[:, c, :, :] for c in range(4)], wa_sb, 512, O * 10
            )
            for mc, ptile in transpose_rows(rows_l3, O * 10, 512):
                nc.vector.tensor_scalar_add(
                    out=l3T[:, mc, :, :], in0=ptile[:, :], scalar1=bcol(12 + mc)
                )

            # union softmax + weighted reduce + invert -> catF chunks 0..7
            # (batched: [P, 4, O, 10] at once)
            catF = act.tile([P, 32, O], F32)
            mx3 = tmp.tile([P, 4, O], F32, tag="mx3")
            nc.vector.reduce_max(mx3, l3T[:, :, :, :], axis=AX)
            d3 = tmp.tile([P, 4, O, 10], F32, tag="d3")
            nc.vector.tensor_tensor(
                out=d3,
                in0=l3T[:, :, :, :],
                in1=mx3.broadcast_to([P, 4, O, 10]),
                op=OP.subtract,
            )
            w3 = tmp.tile([P, 4, O, 10], F32, tag="w3")
            nc.scalar.activation(out=w3, in_=d3, func=AF.Exp)
            s3 = tmp.tile([P, 4, O], F32, tag="s3")
            nc.vector.reduce_sum(s3, w3, axis=AX)
            r3 = tmp.tile([P, 4, O], F32, tag="r3")
            nc.vector.reciprocal(out=r3, in_=s3)
            wn3 = tmp.tile([P, 4, O, 10], F32, tag="wn3")
            nc.vector.tensor_tensor(
                out=wn3, in0=w3, in1=r3.broadcast_to([P, 4, O, 10]), op=OP.mult
            )
            for half, src in ((0, raT), (1, rbT)):
                tu = tmp.tile([P, 4, O, 10], F32, tag="tu")
                nc.vector.tensor_tensor(
                    out=tu, in0=wn3, in1=src[:, :, :, :], op=OP.mult
                )
                su = tmp.tile([P, 4, O], F32, tag="su")
                nc.vector.reduce_sum(su, tu, axis=AX)
                nc.vector.reciprocal(
                    out=catF[:, half * 4 : half * 4 + 4, :], in_=su
                )

            # catF chunks 8..31: a_ac, b_ac, a_o, b_o, a_q, b_q from eT
            for j, (half, k) in enumerate(
                ((0, 12), (1, 12), (0, 11), (1, 11), (0, 10), (1, 10))
            ):
                nc.gpsimd.tensor_copy(
                    out=catF[:, 8 + j * 4 : 12 + j * 4, :],
                    in_=eT[:, half * 4 : half * 4 + 4, :, k],
                )

            # ---- classify head (fp32): hf = cat @ Wl0, rows [O, 512]
            pf = prow.tile([O, 512], F32, tag="prow")
            for kc in range(32):
                nc.tensor.matmul(
                    out=pf,
                    lhsT=catF[:, kc, :],
                    rhs=wl0_sb[:, kc, :],
                    start=(kc == 0),
                    stop=(kc == 31),
                )
            # out = relu(hf + bl0) . Wl + bl, all on the vector engine
            hrelu = rowsp.tile([O, 512], F32, tag="hrelu")
            nc.vector.tensor_tensor(out=hrelu, in0=pf[:, :], in1=bl0r_sb, op=OP.add)
            nc.vector.tensor_scalar_max(out=hrelu, in0=hrelu, scalar1=0.0)
            hw = rowsp.tile([O, 512], F32, tag="hw")
            nc.vector.tensor_tensor(out=hw, in0=hrelu, in1=wlr_sb, op=OP.mult)
            osum = rowsp.tile([O, 1], F32, tag="osum")
            nc.vector.reduce_sum(osum, hw, axis=AX)
            out_sb = rowsp.tile([O, 1], F32, tag="out_sb")
            nc.vector.tensor_scalar_add(
                out=out_sb, in0=osum, scalar1=bias_sb[0:O, 20:21]
            )
            nc.sync.dma_start(out=out_d[:], in_=out_sb)

            if debug:
                for name, t in (
                    ("xT", xT),
                    ("eT", eT),
                    ("l1T", l1T),
                    ("cat2", cat2),
                    ("raT", raT),
                    ("rbT", rbT),
                    ("catF", catF),
                ):
                    dt = F32 if t is not xT else BF16
                    d = nc.dram_tensor(
                        "dbg_" + name, list(t.shape), dt, kind="ExternalOutput"
                    )
                    nc.sync.dma_start(out=d[:], in_=t)

    _split_excess_waits(nc)
    return nc


_NC = None


def _get_nc():
    global _NC
    if _NC is None:
        _NC = _build_nc()
    return _NC


def _prep_inputs(hidden, idx, Wp, bp, Wa0, ba0, Wa, ba, Wl0, bl0, Wl, bl):
    hidden = np.asarray(hidden, dtype=np.float32)
    idx = np.asarray(idx).astype(np.int64)

    f32 = lambda a: np.ascontiguousarray(np.asarray(a, dtype=np.float32))
    bf = lambda a: np.ascontiguousarray(np.asarray(a, dtype=np.float32).astype(NPBF16))
    bp, ba0, ba, bl0, bl = f32(bp), f32(ba0), f32(ba), f32(bl0), f32(bl)
    Wl = f32(Wl)

    hid_b = np.ascontiguousarray(hidden.astype(NPBF16))  # [B, O, L, E]
    wp_t = bf(np.asarray(Wp, np.float32).reshape(8, P, 1024).transpose(1, 0, 2))
    wa0_t = bf(np.asarray(Wa0, np.float32).reshape(8, P, 512).transpose(1, 0, 2))
    wa_t = bf(np.asarray(Wa, np.float32).reshape(4, P, 512).transpose(1, 0, 2))
    wl0_t = f32(
        np.asarray(Wl0, np.float32).reshape(32, P, 512).transpose(1, 0, 2)
    )

    biases = np.zeros((P, 21), dtype=np.float32)
    biases[:, 0:8] = (bp + 1.0).reshape(8, P).T
    biases[:, 8:12] = ba0.reshape(4, P).T
    biases[:, 12:16] = ba.reshape(4, P).T
    biases[:, 16:20] = bl0.reshape(4, P).T
    biases[:, 20] = bl[0]

    bl0rep = np.ascontiguousarray(np.broadcast_to(bl0, (O, 512)).astype(np.float32))
    wlrep = np.ascontiguousarray(np.broadcast_to(Wl[:, 0], (O, 512)).astype(np.float32))

    in_maps = []
    for b in range(B):
        m = np.zeros((L, NK), dtype=np.float32)
        cntinv = np.zeros((NK, 1), dtype=np.float32)
        ib = idx[b]
        starts = [1] + [int(ib[k]) for k in range(9)]
        ends = [int(ib[k]) for k in range(10)]
        segs = [(starts[k], ends[k]) for k in range(10)]
        segs.append((int(ib[9]), int(ib[10])))
        segs.append((int(ib[10]), int(ib[11])))
        segs.append((1, int(ib[9])))
        for k, (s, e) in enumerate(segs):
            m[s:e, k] = 1.0
            cntinv[k, 0] = 1.0 / (e - s)
        maskt = np.ascontiguousarray(
            m.reshape(T, P, NK).transpose(1, 0, 2).astype(NPBF16)
        )

        in_maps.append(
            dict(
                hidden=np.ascontiguousarray(hid_b[b]),
                maskt=maskt,
                cntinv=cntinv,
                wp=wp_t,
                wa0=wa0_t,
                wa=wa_t,
                wl0=wl0_t,
                biases=biases,
                bl0rep=bl0rep,
                wlrep=wlrep,
            )
        )
    return in_maps


def _run(in_maps, **kwargs):
    return run_bass_kernel_spmd(_get_nc(), in_maps, core_ids=list(range(B)), **kwargs)


def kernel(**inputs):
    in_maps = _prep_inputs(**inputs)
    res = _run(in_maps)
    return np.stack([r["out"].reshape(O, 1) for r in res.results])


def _install_ntff_hook():
    """The RL container's antenv lacks axon_hooks, so boot() skipped NTFF
    hook registration. Recreate the module and register the ctypes hook."""
    import sys
    import types

    name = "antenv.axon_hooks"
    if name not in sys.modules:
        try:
            __import__(name)
        except ImportError:
            mod = types.ModuleType(name)
            mod._hook = None
            mod.set_axon_ntff_profile_hook = lambda h: setattr(mod, "_hook", h)
            mod.get_axon_ntff_profile_hook = lambda: mod._hook
            sys.modules[name] = mod
            import antenv

            antenv.axon_hooks = mod
    import antenv.axon_hooks as ah

    if ah.get_axon_ntff_profile_hook() is None:
        from trn_agent_boot.trn_boot import _ntff_profile_via_ctypes

        ah.set_axon_ntff_profile_hook(
            _ntff_profile_via_ctypes("/opt/axon/libaxon_pjrt.so")
        )

    import concourse.bass_utils as bu

    bu.upload_artifacts = lambda tmpdir: tmpdir


def benchmark(trace_cores=None, **inputs):
    """Run with NTFF tracing; returns (output, BassKernelResults)."""
    _install_ntff_hook()
    in_maps = _prep_inputs(**inputs)
    res = _run(in_maps, trace=True, trace_cores=trace_cores)
    out = np.stack([r["out"].reshape(O, 1) for r in res.results])
    return out, res
